# revision 24
# baseline (speedup 1.0000x reference)
import sys
sys.path.insert(0, "/opt/trn_rl_repo")
import numpy as np
import concourse.bass as bass
import concourse.mybir as mybir
import concourse.tile as tile
from concourse import bacc
from concourse.bass_utils import run_bass_kernel_spmd
from concourse.masks import make_identity

F32 = mybir.dt.float32
BF16 = mybir.dt.bfloat16
AF = mybir.ActivationFunctionType
OP = mybir.AluOpType
AX = mybir.AxisListType

S = 2048          # sequence length
H = 4096          # hidden dim
DH = 128          # head dim
NQ = 4            # q heads per core (32 / 8)
NT = S // 128     # 16 q tiles of 128
NCORES = 8
SCALE = 1.0 / np.sqrt(128.0)
NEG = -1.0e33

_CACHED = {}


def _phase_b(nc, tc, hidT_d, wqkvT_d, cos_d, sin_d, qT, kT, vT, cosb, sinb,
             identf, emit_vnat):
    # kt-major: for each 512-col seq chunk, stream hid k-tiles and issue all
    # 6 output-row matmuls per k-tile so PE work starts as soon as the first
    # (w, hid) tile pair lands instead of after a full chunk of DMA.
    with tc.tile_pool(name="wqp", bufs=1) as wq_p, \
         tc.tile_pool(name="hid", bufs=2) as hid_p, \
         tc.tile_pool(name="bps", bufs=1, space="PSUM") as bps, \
         tc.tile_pool(name="stg", bufs=1) as stg_p, \
         tc.tile_pool(name="rt", bufs=2) as rt_p:
        wqs = []
        for kt in range(32):
            w = wq_p.tile([128, 768], BF16, tag=f"wq{kt}")
            nc.sync.dma_start(w, wqkvT_d[kt * 128:(kt + 1) * 128, :])
            wqs.append(w)
        nc.sync.dma_start(cosb, cos_d[:, :])
        nc.sync.dma_start(sinb, sin_d[:, :])

        warmed = False
        for sc in range(4):
            ssl = slice(sc * 512, (sc + 1) * 512)
            pss = [bps.tile([128, 512], F32, tag=f"bacc{m}", name=f"bacc{m}")
                   for m in range(6)]
            for kt in range(32):
                ht = hid_p.tile([128, 512], BF16, tag=f"h{kt % 8}")
                nc.gpsimd.dma_start(ht, hidT_d[kt * 128:(kt + 1) * 128, ssl])
                for m in range(6):
                    nc.tensor.matmul(
                        pss[m], wqs[kt][:, m * 128:(m + 1) * 128], ht,
                        start=(kt == 0), stop=(kt == 31))
                if not warmed and kt == 2:
                    # pre-warm exp table early so the ~1.3us table load is
                    # off the phase-C critical path
                    warm = rt_p.tile([128, 1], BF16, tag="warm")
                    nc.scalar.activation(warm, identf[:, 0:1], AF.Exp)
                    warmed = True
            # drain: free the 6 PSUM banks fast by staging to SBUF with the
            # rotate_half swap pre-applied (ACT does the cross-partition
            # half copies from PSUM, DVE the straight copy); rope math then
            # runs SBUF-only off the critical bank-reuse path. The v-natural
            # transposes (and their ACT copies) are emitted right after the
            # last vT chunk so attention's PV dependencies resolve early.
            nc.scalar.copy(vT[:, ssl], pss[5])
            if sc == 3:
                emit_vnat()
            sgs = []
            for m in range(5):
                sg = stg_p.tile([128, 512], F32, tag=f"sg{m}", name=f"sg{m}")
                sw = stg_p.tile([128, 512], F32, tag=f"sw{m}", name=f"sw{m}")
                if sc == 3 and m % 2 == 1:
                    # last chunk: shift 2 of 5 half-copy pairs off ACT so
                    # the first attention exps start ~1.4us sooner
                    nc.vector.tensor_copy(sw[0:64], pss[m][64:128])
                    nc.vector.tensor_copy(sw[64:128], pss[m][0:64])
                else:
                    nc.scalar.copy(sw[0:64], pss[m][64:128])
                    nc.scalar.copy(sw[64:128], pss[m][0:64])
                nc.vector.tensor_copy(sg, pss[m])
                sgs.append((sg, sw))
            for m in range(5):
                dst = qT[:, m, ssl] if m < 4 else kT[:, ssl]
                sg, sw = sgs[m]
                t1 = rt_p.tile([128, 512], F32, tag="t1")
                t2 = rt_p.tile([128, 512], F32, tag="t2")
                nc.vector.tensor_mul(t1, sg, cosb[:, ssl])
                nc.vector.tensor_mul(t2, sw, sinb[:, ssl])
                nc.vector.tensor_add(dst, t1, t2)


def _phase_cd(nc, tc, qT, kT, vT, vnat, attnT, identb, ones_sq, wogs, out_d,
              tr_cm, tr_p):
    # Transposed-score attention fused with o_proj:
    #   st^T[k, (h,q)] = k_tile^T @ q_4heads in one N=512 matmul; exp(st^T)
    #   is directly the PV rhs (no PE transposes). Denominator via an
    #   all-ones [128,128] stationary matmul (row sums replicated across all
    #   partitions). Normalization folded into the PV drain.
    #   o_proj groups are interleaved into the attention step stream as
    #   their seq-chunks finalize, filling PE idle while ACT runs exp.
    tr_cm.__exit__(None, None, None)

    with tc.tile_pool(name="stp", bufs=2, space="PSUM") as st_p, \
         tc.tile_pool(name="pvp", bufs=2, space="PSUM") as pv_p, \
         tc.tile_pool(name="dnp", bufs=2, space="PSUM") as dn_p, \
         tc.tile_pool(name="dps", bufs=2, space="PSUM") as dps, \
         tc.tile_pool(name="ptp", bufs=4) as pt_p, \
         tc.tile_pool(name="rcp", bufs=2) as rc_p, \
         tc.tile_pool(name="ob", bufs=4) as ob_p:
        pv_cur = {}
        dn_cur = {}

        def make_pvden(t, kb, pt, first, last):
            def th():
                if first:
                    pv_cur[t] = pv_p.tile([128, NQ, 128], F32, tag="pv",
                                          name="pv")
                    dn_cur[t] = dn_p.tile([128, NQ, 128], F32, tag="dn",
                                          name="dn")
                nc.tensor.matmul(pv_cur[t], vnat[:, kb, :], pt,
                                 start=first, stop=last)
                nc.tensor.matmul(dn_cur[t], ones_sq, pt,
                                 start=first, stop=last)
            return th

        def make_fin(t):
            def th():
                rc = rc_p.tile([128, NQ, 128], F32, tag="rc")
                nc.vector.reciprocal_approx_fast(rc, dn_cur[t])
                nc.vector.tensor_mul(
                    attnT[:, :, t * 128:(t + 1) * 128], pv_cur[t], rc)
            return th

        def emit_d_group(m, c0, cw, in_c):
            mg, mo = divmod(m, 4)
            po = dps.tile([128, 512], F32, tag="po", name="po")
            for a in range(NQ):
                nc.tensor.matmul(
                    po[:, 0:cw], wogs[mg][:, a, mo * 128:(mo + 1) * 128],
                    attnT[:, a, c0:c0 + cw],
                    start=(a == 0), stop=(a == NQ - 1))
            ob = ob_p.tile([128, 512], BF16, tag="ob", name="ob")
            if in_c or (m % 2 == 0):
                nc.vector.tensor_copy(ob[:, 0:cw], po[:, 0:cw])
            else:
                nc.scalar.copy(ob[:, 0:cw], po[:, 0:cw])
            nc.sync.dma_start(
                out_d[m * 128:(m + 1) * 128, c0:c0 + cw], ob[:, 0:cw])

        # diag block first within each t: its gpsimd prob-zeroing latency
        # hides behind the remaining kb steps instead of delaying fin(t)
        steps = []
        for t in range(NT):
            kbs = [t] + list(range(t))
            for j, kb in enumerate(kbs):
                steps.append((t, kb, j == 0, j == len(kbs) - 1))

        d_groups = [(m, scc * 512, 512, 4 * scc + 3)
                    for scc in range(4) for m in range(32)]
        fin_step = {}
        n_d = 0
        deferred = {}
        for i, (t, kb, first, last) in enumerate(steps):
            st = st_p.tile([128, NQ, 128], F32, tag="st")
            nc.tensor.matmul(st, kT[:, kb * 128:(kb + 1) * 128],
                             qT[:, :, t * 128:(t + 1) * 128],
                             start=True, stop=True)
            pt = pt_p.tile([128, NQ, 128], BF16, tag="pt")
            nc.scalar.activation(pt, st, AF.Exp, scale=SCALE)
            if kb == t:
                # zero the strictly-upper (q < k) probs of the diagonal
                # block: equivalent to the NEG causal mask, off the DVE path
                ptm = pt_p.tile([128, NQ, 128], BF16, tag="ptm")
                nc.gpsimd.affine_select(
                    out=ptm, in_=pt, pattern=[[0, NQ], [1, 128]],
                    compare_op=OP.is_ge, fill=0.0,
                    base=0, channel_multiplier=-1)
                pt = ptm
            for th in deferred.pop(i, []):
                th()
            deferred.setdefault(i + 1, []).append(
                make_pvden(t, kb, pt, first, last))
            if last:
                deferred.setdefault(i + 1, []).append(make_fin(t))
                fin_step[t] = i + 1
            # interleave ready o_proj groups, one per step to keep the
            # PE stream smooth
            for _ in range(1):
                if n_d >= len(d_groups):
                    break
                m, c0, cw, gate = d_groups[n_d]
                fs = fin_step.get(gate)
                if fs is None or i < fs + 2:
                    break
                emit_d_group(m, c0, cw, True)
                n_d += 1
        for i in sorted(deferred):
            for th in deferred[i]:
                th()
        while n_d < len(d_groups):
            m, c0, cw, gate = d_groups[n_d]
            emit_d_group(m, c0, cw, False)
            n_d += 1


def _build_nc():
    nc = bacc.Bacc(None, target_bir_lowering=False, debug=False)
    # Inputs host-pre-transposed/cast so no PE transposes are needed:
    #   hidt  = hidden[0].T            [H, S]
    #   wqkvt = [Wq_c; Wk_c; Wv_c].T   [H, 768]   (cols 0:512 q, 512:640 k, 640:768 v)
    #   wot   = Wo[:, c*512:...].T     [512, H]
    #   cos/sin [d=128, S], sin sign-folded for rotate_half
    hidT_d = nc.dram_tensor("hidt", [H, S], BF16, kind="ExternalInput")
    wqkvT_d = nc.dram_tensor("wqkvt", [H, 768], BF16, kind="ExternalInput")
    woT_d = nc.dram_tensor("wot", [NQ * DH, H], BF16, kind="ExternalInput")
    cos_d = nc.dram_tensor("cos", [DH, S], BF16, kind="ExternalInput")
    sin_d = nc.dram_tensor("sin", [DH, S], BF16, kind="ExternalInput")
    out_d = nc.dram_tensor("outt", [H, S], BF16, kind="ExternalOutput")

    with tile.TileContext(nc) as tc:
        with tc.tile_pool(name="perm", bufs=1) as perm:
            identf = perm.tile([128, 128], F32, tag="identf")
            make_identity(nc, identf)
            identb = perm.tile([128, 128], BF16, tag="identb")
            nc.vector.tensor_copy(identb, identf)
            ones_sq = perm.tile([128, 128], BF16, tag="ones_sq")
            nc.gpsimd.memset(ones_sq, 1.0)

            # persistent strips (bf16)
            qT = perm.tile([128, NQ, S], BF16, tag="qT")
            kT = perm.tile([128, S], BF16, tag="kT")
            vT = perm.tile([128, S], BF16, tag="vT")
            vnat = perm.tile([128, NT, 128], BF16, tag="vnat")
            attnT = perm.tile([128, NQ, S], BF16, tag="attnT")
            cosb = perm.tile([128, S], BF16, tag="cosb")
            sinb = perm.tile([128, S], BF16, tag="sinb")

            # open the v-transpose PSUM pool BEFORE phase B's accumulator
            # pool so it gets disjoint banks and the transposes never wait
            # on the last rope drain
            tr_cm = tc.tile_pool(name="tps", bufs=2, space="PSUM")
            tr_p = tr_cm.__enter__()

            def emit_vnat():
                for g in range(2):
                    tp = tr_p.tile([128, 8, 128], BF16, tag="tp", name="tp")
                    for i in range(8):
                        st8 = 8 * g + i
                        nc.tensor.transpose(
                            tp[:, i, :], vT[:, st8 * 128:(st8 + 1) * 128],
                            identb)
                    nc.scalar.copy(vnat[:, 8 * g:8 * g + 8, :], tp)

            _phase_b(nc, tc, hidT_d, wqkvT_d, cos_d, sin_d, qT, kT, vT,
                     cosb, sinb, identf, emit_vnat)

            # o_proj weights: load early on the (now idle) sync queue
            with tc.tile_pool(name="wo", bufs=1) as wo_p:
                wogs = []
                for mg in range(8):
                    wg = wo_p.tile([128, NQ, 512], BF16, tag=f"wo{mg}")
                    for a in range(NQ):
                        nc.sync.dma_start(
                            wg[:, a, :],
                            woT_d[a * 128:(a + 1) * 128, mg * 512:(mg + 1) * 512])
                    wogs.append(wg)

                _phase_cd(nc, tc, qT, kT, vT, vnat, attnT, identb, ones_sq,
                          wogs, out_d, tr_cm, tr_p)
    nc.compile()
    return nc


def _prep_inputs(hidden_states, position_ids, Wq, Wk, Wv, Wo):
    bf16 = np.dtype(mybir.dt.np(BF16))
    hs = np.asarray(hidden_states, dtype=np.float32)
    hidT = np.ascontiguousarray(hs[0].T).astype(bf16)

    pos = np.asarray(position_ids).reshape(-1).astype(np.float64)
    invf = 1.0 / (10000.0 ** (np.arange(0, 128, 2, dtype=np.float64) / 128.0))
    ang = invf[:, None] * pos[None, :]
    cos_t = np.concatenate([np.cos(ang), np.cos(ang)], axis=0).astype(bf16)
    sin_t = np.concatenate([-np.sin(ang), np.sin(ang)], axis=0).astype(bf16)

    Wq = np.asarray(Wq, dtype=np.float32)
    Wk = np.asarray(Wk, dtype=np.float32)
    Wv = np.asarray(Wv, dtype=np.float32)
    Wo = np.asarray(Wo, dtype=np.float32)
    in_maps = []
    for c in range(NCORES):
        wqkv = np.concatenate([
            Wq[c * 512:(c + 1) * 512],
            Wk[c * 128:(c + 1) * 128],
            Wv[c * 128:(c + 1) * 128]], axis=0)          # [768, H]
        wqkvT = np.ascontiguousarray(wqkv.T).astype(bf16)  # [H, 768]
        woT = np.ascontiguousarray(Wo[:, c * 512:(c + 1) * 512].T).astype(bf16)
        in_maps.append({"hidt": hidT, "wqkvt": wqkvT, "wot": woT,
                        "cos": cos_t, "sin": sin_t})
    return in_maps


def kernel(hidden_states, position_ids, Wq, Wk, Wv, Wo, **extra):
    hs = np.asarray(hidden_states)
    B = hs.shape[0]
    assert B == 1 and hs.shape[1] == S and hs.shape[2] == H

    if "nc" not in _CACHED:
        _CACHED["nc"] = _build_nc()
    nc = _CACHED["nc"]

    in_maps = _prep_inputs(hidden_states, position_ids, Wq, Wk, Wv, Wo)
    res = run_bass_kernel_spmd(nc, in_maps, core_ids=list(range(NCORES)))
    out = np.zeros((H, S), dtype=np.float32)
    for c in range(NCORES):
        out += np.asarray(res.results[c]["outt"]).astype(np.float32)
    return np.ascontiguousarray(out.T).reshape(1, S, H)
